# revision 35
# baseline (speedup 1.0000x reference)
"""Trainium2 Bass kernel for a ViT-style dense transformer block (v2).

Reference computation:
    xn = LN1(x); qkv = xn @ Wqkv.T + b; 16-head softmax attention;
    x = x + attn_out @ Wp.T + bp;
    out = x + gelu(LN2(x) @ W1.T + b1) @ W2.T + b2
Shapes: x [16, 577, 1024], heads 16, head_dim 64, hidden 4096.

Sharding: data-parallel over batch across 8 NeuronCores; each core gets 2
batch items concatenated along tokens (T = 2*577 = 1154). No collectives.

Structure (vs the 743us v0 baseline; measured ~606-626us):
  * LayerNorm is restructured so the device only computes
    z = x * rsqrt(var+eps) (cheap bf16 2x-mode DVE ops). gamma/beta are
    folded into the consumer weights on the host; the -mean*r*u + bias
    correction is one extra matmul accumulated into each consumer's PSUM
    group. Its operands are padded to full 128 rows (row 0 = mean*r,
    row 32 = ones, zeros elsewhere) because K=2 operands stall the
    tensor engine's LDWEIGHTS pull-ahead at every PSUM-group boundary
    (measured +200ns per group; padded operands run at the 163ns/MM
    ideal).
  * The QKV dense matmuls are emitted as small quanta interleaved into
    the attention pair loop, so the tensor engine always has dense work
    while the scalar engine streams the softmax exps. This keeps the PE
    HAM clock at 8/8 (the v0 kernel spent 226us throttled at K=4/8).
  * One-pair lookahead: scores+exps for pair p+1 are emitted before
    finish_group(p) so the exp stream never drains; exps for both heads
    of a pair are evaluated by a single ACT instruction over a 4-bank
    score tile.  q/k are stored fp8e4 (softmax averaging washes the
    quantization out of y); full fp8 matmuls were evaluated and rejected
    (e4m3 rms quantization ~3.6% per operand would blow the 2e-2 gate).
  * LN1 runs in four fine front tiles interleaved with pair-0 q/k, v,
    and even pair-0 scores; LN2 tiles 1-2 are deferred underneath fc1's
    tile-0 pass.  Weight streams issue from the otherwise-idle GPSIMD
    queue to avoid sync-queue head-of-line blocking.
  * Softmax denominators use reciprocal_approx_fast (DVE custom op);
    normalization multiplies stashed bf16 y_raw by a rank-1 PE broadcast
    of the reciprocal row.
"""

import math
import numpy as np
import ml_dtypes
from collections import deque
from contextlib import ExitStack

import concourse.bacc as bacc
import concourse.mybir as mybir
import concourse.tile as tile
from concourse.bass_utils import run_bass_kernel_spmd

F32 = mybir.dt.float32
BF16 = mybir.dt.bfloat16
FP8 = mybir.dt.float8e4
AF = mybir.ActivationFunctionType
ALU = mybir.AluOpType

N_CORES = 8
P = 128
C = 1024
KC = C // P          # 8 feature chunks
H = 16               # heads
D = 64               # head dim
HID = 4096
MH = HID // P        # 32 hidden chunks
NTOK = 577           # tokens per item
T = 2 * NTOK         # tokens per core
EPS = 1e-5

# token tiles for dense (non-attention) phases; uniform ~385 keeps every
# matmul free-dim large enough to hide LDWEIGHTS
TT = [(0, 385), (385, 385), (770, 384)]
# query tiles per item (relative to item start)
QT = [(0, 512), (512, 65)]
# key chunks per item: 4 full 128s + one 65
KCH = [(0, 128), (128, 128), (256, 128), (384, 128), (512, 65)]

LAST_EXEC_NS = None
_NC = None


def _build(num_devices=N_CORES):
    nc = bacc.Bacc(
        "TRN2", target_bir_lowering=False, debug=False, num_devices=num_devices
    )

    xT_d = nc.dram_tensor("xT", [P, KC, T], F32, kind="ExternalInput")
    wqk_d = nc.dram_tensor("wqk", [2 * KC, P, KC, P], BF16, kind="ExternalInput")
    wv_d = nc.dram_tensor("wv", [P, KC, C], BF16, kind="ExternalInput")
    wp_d = nc.dram_tensor("wp", [KC, P, KC, P], BF16, kind="ExternalInput")
    w1_d = nc.dram_tensor("w1", [MH, P, KC, P], BF16, kind="ExternalInput")
    w2_d = nc.dram_tensor("w2", [KC, P, MH, P], BF16, kind="ExternalInput")
    ubqk_d = nc.dram_tensor("ubqk", [P, 2 * KC, P], BF16, kind="ExternalInput")
    uvbv_d = nc.dram_tensor("uvbv", [P, C], BF16, kind="ExternalInput")
    ub1_d = nc.dram_tensor("ub1", [P, MH, P], BF16, kind="ExternalInput")
    pb_d = nc.dram_tensor("pb", [P, KC], F32, kind="ExternalInput")
    b2_d = nc.dram_tensor("b2", [P, KC], F32, kind="ExternalInput")
    out_d = nc.dram_tensor("outT", [P, KC, T], F32, kind="ExternalOutput")

    with tile.TileContext(nc) as tc, ExitStack() as top:
        const_p = top.enter_context(tc.tile_pool(name="consts", bufs=1))
        xp = top.enter_context(tc.tile_pool(name="xres", bufs=1))
        zp = top.enter_context(tc.tile_pool(name="zpool", bufs=1))

        pb = const_p.tile([P, KC], F32)
        nc.sync.dma_start(out=pb, in_=pb_d[:])
        b2 = const_p.tile([P, KC], F32)
        nc.sync.dma_start(out=b2, in_=b2_d[:])
        ubqk = const_p.tile([P, 2 * KC, P], BF16)
        nc.sync.dma_start(out=ubqk, in_=ubqk_d[:])
        uvbv = const_p.tile([P, C], BF16)
        nc.sync.dma_start(out=uvbv, in_=uvbv_d[:])
        inv_ones = const_p.tile([P, 1], BF16)
        nc.vector.memset(inv_ones, 1.0 / C)
        ones_pp = const_p.tile([P, P], BF16)  # ones; rank-1 lhsT at any base
        nc.vector.memset(ones_pp, 1.0)
        ones_r1 = ones_pp[0:1, :]

        # residual stream, feature-major f32 (updated in place with x2)
        xT = xp.tile([P, KC, T], F32)
        for t0, tw in ((0, 192), (192, 193), (385, 385), (770, 384)):
            nc.sync.dma_start(
                out=xT[:, :, t0:t0 + tw], in_=xT_d[:, :, t0:t0 + tw]
            )

        # z = x * rsqrt(var+eps) in bf16; rows2 = [mean*r ; ones] for the
        # rank-1 LN correction matmuls. z is reused for LN2's output too.
        z = zp.tile([P, KC, T], BF16)
        # padded correction rows: row 0 = mean*r (per tile), row 32 = ones,
        # the rest stay zero -- full-128-row lhsT/rhs keeps LDWEIGHTS
        # pull-ahead working at PSUM-group boundaries (K=2 operands stall it)
        rows2a = zp.tile([P, T], BF16)
        rows2b = zp.tile([P, T], BF16)
        nc.vector.memset(rows2a, 0.0)
        nc.vector.memset(rows2b, 0.0)
        nc.vector.memset(rows2a[32:33, :], 1.0)
        nc.vector.memset(rows2b[32:33, :], 1.0)

        def ln_tile(src, rows2, t0, tw, ln_sb, ln_ps, rb_ps=None, rb_tag="lnrb"):
            if rb_ps is None:
                rb_ps = ln_ps
            """One token tile of cheap LN: z[:, :, ts] = src*rsqrt(var+eps),
            rows2[0, ts] = mean*r."""
            ts = slice(t0, t0 + tw)
            for ci in range(KC):
                nc.scalar.copy(z[:, ci, ts], src[:, ci, ts])
            s1 = ln_ps.tile([1, 512], F32, tag="s1", name="s1", bufs=1)[:, :tw]
            s2 = ln_ps.tile([1, 512], F32, tag="s2", name="s2", bufs=1)[:, :tw]
            for ci in range(KC):
                xsq = ln_sb.tile([P, 512], BF16, tag="xsq", name="xsq", bufs=2)[:, :tw]
                nc.vector.tensor_mul(xsq, z[:, ci, ts], z[:, ci, ts])
                nc.tensor.matmul(
                    s1, inv_ones, z[:, ci, ts], start=(ci == 0), stop=(ci == KC - 1)
                )
                nc.tensor.matmul(
                    s2, inv_ones, xsq, start=(ci == 0), stop=(ci == KC - 1)
                )
            msq = ln_sb.tile([1, 512], F32, tag="msq", name="msq", bufs=1)[:, :tw]
            nc.scalar.square(msq, s1)
            veps = ln_sb.tile([1, 512], F32, tag="veps", name="veps", bufs=1)[:, :tw]
            nc.vector.scalar_tensor_tensor(veps, s2, EPS, msq, ALU.add, ALU.subtract)
            rinv = ln_sb.tile([1, 512], F32, tag="rinv", name="rinv", bufs=1)[:, :tw]
            nc.vector.reciprocal_approx_fast(rinv, veps)
            rbf = ln_sb.tile([1, 512], BF16, tag="rbf", name="rbf", bufs=1)[:, :tw]
            nc.scalar.activation(rbf, rinv, AF.Sqrt)
            nc.vector.tensor_mul(rows2[0:1, ts], s1, rbf)
            ps_rb = rb_ps.tile([P, 512], F32, tag=rb_tag, name="lnrb", bufs=2)[:, :tw]
            nc.tensor.matmul(ps_rb, ones_r1, rbf, start=True, stop=True)
            rb = ln_sb.tile([P, 512], BF16, tag="rb", name="rb", bufs=2)[:, :tw]
            nc.scalar.copy(rb, ps_rb)
            for ci in range(KC):
                nc.vector.tensor_mul(z[:, ci, ts], z[:, ci, ts], rb)

        # ---------------- attention + interleaved QKV ----------------
        attn_scope = top.enter_context(ExitStack())
        yp = attn_scope.enter_context(tc.tile_pool(name="ypool", bufs=1))
        yT = yp.tile([P, KC, T], BF16)  # attention output (pre-proj)

        # everything else attention-internal closes before proj
        inner = ExitStack()
        az = inner.enter_context(tc.tile_pool(name="azpool", bufs=1))
        wv_pool = inner.enter_context(tc.tile_pool(name="wvp", bufs=1))
        wqk_pool = inner.enter_context(tc.tile_pool(name="wqkp", bufs=3))
        at_sb = inner.enter_context(tc.tile_pool(name="atsb", bufs=2))

        # q/k live in fp8e4: the softmax averages ~500 weights, so per-score
        # quantization noise washes out of y; halves the qkT footprint
        qkT = az.tile([P, 2 * KC, T], FP8)
        v_aug = az.tile([P, 10, H, 65], BF16)
        nc.vector.memset(v_aug[:, :, :, 64:65], 1.0)

        wv = wv_pool.tile([P, KC, C], BF16)
        nc.gpsimd.dma_start(out=wv, in_=wv_d[:])

        s_attn = ExitStack()
        mm_ps = s_attn.enter_context(
            tc.tile_pool(name="mmps", bufs=2, space="PSUM")
        )

        def emit_qk_tile(m, wqkb, t0, tw):
            ts = slice(t0, t0 + tw)
            ps = mm_ps.tile([P, 512], F32, tag="mm", name="qk", bufs=2)[:, :tw]
            nc.tensor.matmul(
                ps, ubqk[:, m, :], rows2a[:, ts], start=True, stop=False
            )
            for ci in range(KC):
                nc.tensor.matmul(
                    ps, wqkb[:, ci, :], z[:, ci, ts],
                    start=False, stop=(ci == KC - 1),
                )
            nc.scalar.copy(qkT[:, m, ts], ps)

        def qk_quanta(m):
            state = {}

            def mk(t_idx):
                def run():
                    if "w" not in state:
                        w = wqk_pool.tile([P, KC, P], BF16, tag="wqkb", name="wqkb")
                        nc.gpsimd.dma_start(out=w, in_=wqk_d[m])
                        state["w"] = w
                    emit_qk_tile(m, state["w"], *TT[t_idx])
                return run

            return [mk(i) for i in range(3)]

        def v_quantum(it, half, c_i):
            def run():
                k0, kw = KCH[c_i]
                tok0 = it * NTOK + k0
                tks = slice(tok0, tok0 + kw)
                hs = slice(half * 512, (half + 1) * 512)
                ps = mm_ps.tile([P, 512], F32, tag="mm", name="v", bufs=2)
                nc.tensor.matmul(
                    ps[:kw, :], rows2a[:, tks], uvbv[:, hs],
                    start=True, stop=False,
                )
                for ci in range(KC):
                    nc.tensor.matmul(
                        ps[:kw, :], z[:, ci, tks], wv[:, ci, hs],
                        start=False, stop=(ci == KC - 1),
                    )
                dst = v_aug[:kw, it * 5 + c_i, half * 8:(half + 1) * 8, 0:64]
                nc.vector.tensor_copy(
                    out=dst, in_=ps[:kw].rearrange("p (h d) -> p h d", d=64)
                )
            return run


        # dense quanta fed into the attention pair loop.  need = checkpoint
        # index by which the quantum must have been emitted (2p = before
        # pair p scores, 2p+1 = before pair p attn@v).
        quanta = deque()
        for p, need in ((1, 2), (2, 4), (3, 6)):
            for q in qk_quanta(p) + qk_quanta(KC + p):
                quanta.append((need, q))
        for c_i in range(5):
            quanta.append((8, v_quantum(0, 1, c_i)))
        for q in qk_quanta(4) + qk_quanta(KC + 4):
            quanta.append((8, q))
        for c_i in range(5):
            quanta.append((9, v_quantum(1, 1, c_i)))
        for p, need in ((5, 10), (6, 12), (7, 14)):
            for q in qk_quanta(p) + qk_quanta(KC + p):
                quanta.append((need, q))

        budget = [0]
        cur = [0]

        def drain(k):
            while quanta and quanta[0][0] <= k:
                quanta.popleft()[1]()

        def feed():
            # hold back quanta needed >2 pairs out so the tail pairs keep
            # dense work; mid-loop pairs are PE-bound anyway
            if budget[0] > 0 and quanta and quanta[0][0] <= 2 * cur[0] + 4:
                quanta.popleft()[1]()
                budget[0] -= 1

        def emit_scores(it, pair):
            """Scores + exp for both heads of a pair; one exp instruction per
            key chunk covers both heads ([kw, 2, 577] strided over the 4-bank
            score tile)."""
            exps = at_sb.tile(
                [P, 2, 5, NTOK], BF16, tag="exps", name="exps", bufs=3
            )
            for c_i, (k0, kw) in enumerate(KCH):
                ks = slice(it * NTOK + k0, it * NTOK + k0 + kw)
                # hh stride of 1024 fp32 = 2 banks keeps every matmul output
                # inside a single PSUM bank
                ps_s = mm_ps.tile(
                    [P, 2, 1024], F32, tag="score", name="score", bufs=1
                )
                for hh in range(2):
                    rows = slice(hh * D, (hh + 1) * D)
                    for q0, qw in QT:
                        qs = slice(it * NTOK + q0, it * NTOK + q0 + qw)
                        nc.tensor.matmul(
                            ps_s[:kw, hh, q0:q0 + qw],
                            qkT[rows, KC + pair, ks],
                            qkT[rows, pair, qs],
                            start=True, stop=True,
                        )
                nc.scalar.activation(
                    exps[:kw, :, c_i, :], ps_s[:kw, :, :NTOK],
                    AF.Exp, scale=0.125,
                )
                feed()
            return exps


        def emit_attnv(it, pair, exps, coll):
            yraws = []
            for q0, qw in QT:
                for hh in range(2):
                    h = 2 * pair + hh
                    base = 32 * (2 * it + hh)
                    ps_y = at_ps.tile(
                        [P, 512], F32, tag="y", name="y", bufs=2
                    )[:65, :qw]
                    for c_i, (k0, kw) in enumerate(KCH):
                        nc.tensor.matmul(
                            ps_y,
                            v_aug[:kw, it * 5 + c_i, h, :],
                            exps[:kw, hh, c_i, q0:q0 + qw],
                            start=(c_i == 0), stop=(c_i == 4),
                        )
                    nc.vector.tensor_copy(
                        out=coll[base:base + 1, q0:q0 + qw], in_=ps_y[64:65, :]
                    )
                    if q0 == 0:
                        yr = at_sb.tile(
                            [D, NTOK], BF16, tag="yraw", name="yraw", bufs=4
                        )
                        yraws.append(yr)
                    else:
                        yr = yraws[hh]
                    nc.vector.tensor_copy(out=yr[:, q0:q0 + qw], in_=ps_y[0:D, :])
                    feed()
            return yraws

        def finish_group(pair, coll, yraws2):
            recf = at_sb.tile([97, NTOK], F32, tag="recf", name="recf", bufs=1)
            nc.vector.reciprocal_approx_fast(recf, coll)
            recbf = at_sb.tile([97, NTOK], BF16, tag="recbf", name="recbf", bufs=1)
            # DVE, not ACT: keeps the exp stream free of this dependency
            nc.vector.tensor_copy(out=recbf, in_=recf)
            for it in range(2):
                for hh in range(2):
                    base = 32 * (2 * it + hh)
                    for q0, qw in QT:
                        qs = slice(it * NTOK + q0, it * NTOK + q0 + qw)
                        ps_rb = at_ps.tile(
                            [P, 512], F32, tag="y", name="rb", bufs=2
                        )[:D, :qw]
                        nc.tensor.matmul(
                            ps_rb,
                            ones_pp[base:base + 1, 0:D],
                            recbf[base:base + 1, q0:q0 + qw],
                            start=True, stop=True,
                            tile_position=(base, 0),
                        )
                        nc.vector.tensor_tensor(
                            out=yT[hh * D:(hh + 1) * D, pair, qs],
                            in0=yraws2[it][hh][:, q0:q0 + qw],
                            in1=ps_rb, op=ALU.mult,
                        )

        # front: LN1 tiles interleaved with pair-0 q/k, v(half0) and even
        # pair-0 scores, so both PE and ACT ramp as early as possible
        wqk0 = {}
        for m in (0, KC):
            wqk0[m] = wqk_pool.tile([P, KC, P], BF16, tag="wqkb", name="wqkb")
            nc.gpsimd.dma_start(out=wqk0[m], in_=wqk_d[m])
        FTT = [(0, 192), (192, 193), (385, 385), (770, 384)]
        front_v = {0: [(0, 0, 0)],
                   1: [(0, 0, 1), (0, 0, 2)],
                   2: [(0, 0, 3), (0, 0, 4), (1, 0, 0)],
                   3: [(1, 0, 1), (1, 0, 2), (1, 0, 3), (1, 0, 4)]}
        exps_cur = [None, None]
        with ExitStack() as lnx:
            ln_sb = lnx.enter_context(tc.tile_pool(name="ln1sb", bufs=2))
            ln_ps = lnx.enter_context(
                tc.tile_pool(name="ln1ps", bufs=1, space="PSUM")
            )
            for t_idx, (t0, tw) in enumerate(FTT):
                ln_tile(xT, rows2a, t0, tw, ln_sb, ln_ps,
                        rb_ps=mm_ps, rb_tag="mm")
                for m in (0, KC):
                    emit_qk_tile(m, wqk0[m], t0, tw)
                for it, half, c_i in front_v[t_idx]:
                    v_quantum(it, half, c_i)()
                if t_idx == 2:
                    # item0 spans tokens 0..577 < 770, so its pair-0
                    # scores only need z/qkT through token 770
                    exps_cur[0] = emit_scores(0, 0)
            exps_cur[1] = emit_scores(1, 0)

        at_ps = s_attn.enter_context(
            tc.tile_pool(name="atps", bufs=1, space="PSUM")
        )

        # one-pair lookahead: scores+exps for pair p+1 are emitted before
        # finish_group(p), so the scalar engine's exp stream never drains
        budget[0] = max(1, math.ceil(len(quanta) / 8))
        for pair in range(H // 2):
            drain(2 * pair + 1)
            coll = at_sb.tile([97, NTOK], F32, tag="coll", name="coll", bufs=2)
            # only rows at 32-offsets get written; the reciprocal reads all 97
            nc.vector.memset(coll, 1.0)
            yr0 = emit_attnv(0, pair, exps_cur[0], coll)
            yr1 = emit_attnv(1, pair, exps_cur[1], coll)
            if pair < H // 2 - 1:
                drain(2 * pair + 2)
                cur[0] = pair + 1
                budget[0] = max(1, math.ceil(len(quanta) / (7 - pair)))
                exps_cur = [emit_scores(0, pair + 1), emit_scores(1, pair + 1)]
            finish_group(pair, coll, [yr0, yr1])
        while quanta:
            quanta.popleft()[1]()

        s_attn.close()
        inner.close()  # free qkT / v_aug / exps / wv / wqk before proj

        # ---------------- proj + residual + LN2 (per tile) ----------------
        w2_pool = top.enter_context(tc.tile_pool(name="w2pool", bufs=3))
        w2bs = {}
        for m in (0, 1):
            w2bs[m] = w2_pool.tile([P, MH, P], BF16, tag="w2b", name="w2b")
            nc.sync.dma_start(out=w2bs[m], in_=w2_d[m])
        ln2_sb = top.enter_context(tc.tile_pool(name="ln2sb", bufs=2))
        ln2_ps = top.enter_context(
            tc.tile_pool(name="ln2ps", bufs=1, space="PSUM")
        )
        with ExitStack() as pjx:
            pj_ps = pjx.enter_context(
                tc.tile_pool(name="pjps", bufs=2, space="PSUM")
            )
            wp_pool = pjx.enter_context(tc.tile_pool(name="wpp", bufs=3))
            for t0, tw in TT:
                ts = slice(t0, t0 + tw)
                for m in range(KC):
                    wpb = wp_pool.tile([P, KC, P], BF16, tag="wpb", name="wpb")
                    nc.gpsimd.dma_start(out=wpb, in_=wp_d[m])
                    ps = pj_ps.tile([P, 512], F32, tag="z", name="z", bufs=2)[:, :tw]
                    for ci in range(KC):
                        nc.tensor.matmul(
                            ps, wpb[:, ci, :], yT[:, ci, ts],
                            start=(ci == 0), stop=(ci == KC - 1),
                        )
                    # x2 = (z + bp) + x   (in place into xT)
                    nc.vector.scalar_tensor_tensor(
                        xT[:, m, ts], ps, pb[:, m:m + 1], xT[:, m, ts],
                        ALU.add, ALU.add,
                    )
                if t0 == 0:
                    # only tile0's LN2 gates fc1-A; the rest run under it
                    ln_tile(xT, rows2b, t0, tw, ln2_sb, ln2_ps)


        # ------------------------- MLP -------------------------
        with ExitStack() as mx:
            mlpc = mx.enter_context(tc.tile_pool(name="mlpc", bufs=1))
            h_p = mx.enter_context(tc.tile_pool(name="hpool", bufs=1))
            ub1 = mlpc.tile([P, MH, P], BF16)
            nc.gpsimd.dma_start(out=ub1, in_=ub1_d[:])
            hT = h_p.tile([P, MH, T], BF16)

            # fc1 in two tile groups so it can start right after LN2(t0);
            # one shared weight pool with prefetch across the A/B boundary
            with ExitStack() as f1x:
                w1_pool = f1x.enter_context(tc.tile_pool(name="w1pool", bufs=4))
                f1_ps = f1x.enter_context(
                    tc.tile_pool(name="f1ps", bufs=4, space="PSUM")
                )
                steps = [(TT[:1], mh) for mh in range(MH)]
                steps += [(TT[1:], mh) for mh in range(MH)]
                w1bs = {}

                def fetch_w1(i):
                    if i < len(steps) and i not in w1bs:
                        w1bs[i] = w1_pool.tile(
                            [P, KC, P], BF16, tag="w1b", name="w1b"
                        )
                        nc.gpsimd.dma_start(out=w1bs[i], in_=w1_d[steps[i][1]])

                for i, (tgroup, mh) in enumerate(steps):
                    if i == 4:
                        # deferred LN2 tiles run underneath fc1-A
                        ln_tile(xT, rows2b, *TT[1], ln2_sb, ln2_ps)
                    elif i == 8:
                        ln_tile(xT, rows2b, *TT[2], ln2_sb, ln2_ps)
                    fetch_w1(i)
                    fetch_w1(i + 1)
                    fetch_w1(i + 2)
                    w1b = w1bs.pop(i)
                    for t0, tw in tgroup:
                        ts = slice(t0, t0 + tw)
                        ps = f1_ps.tile([P, 512], F32, tag="h", name="h")[:, :tw]
                        nc.tensor.matmul(
                            ps, ub1[:, mh, :], rows2b[:, ts],
                            start=True, stop=False,
                        )
                        for ci in range(KC):
                            nc.tensor.matmul(
                                ps, w1b[:, ci, :], z[:, ci, ts],
                                start=False, stop=(ci == KC - 1),
                            )
                        nc.scalar.activation(hT[:, mh, ts], ps, AF.Gelu)

            with ExitStack() as f2x:
                f2_ps = f2x.enter_context(
                    tc.tile_pool(name="f2ps", bufs=2, space="PSUM")
                )
                o_pool = f2x.enter_context(tc.tile_pool(name="opool", bufs=3))
                for m in range(KC):
                    if m not in w2bs:
                        w2bs[m] = w2_pool.tile([P, MH, P], BF16, tag="w2b", name="w2b")
                        nc.sync.dma_start(out=w2bs[m], in_=w2_d[m])
                    if m + 1 < KC:
                        # prefetch next block so its 1MB DMA hides under MMs
                        w2bs[m + 1] = w2_pool.tile(
                            [P, MH, P], BF16, tag="w2b", name="w2b"
                        )
                        nc.sync.dma_start(out=w2bs[m + 1], in_=w2_d[m + 1])
                    w2b = w2bs[m]
                    for t0, tw in TT:
                        ts = slice(t0, t0 + tw)
                        ps = f2_ps.tile([P, 512], F32, tag="o", name="o")[:, :tw]
                        for kh in range(MH):
                            nc.tensor.matmul(
                                ps, w2b[:, kh, :], hT[:, kh, ts],
                                start=(kh == 0), stop=(kh == MH - 1),
                            )
                        osb = o_pool.tile([P, 512], F32, tag="osb", name="osb")[:, :tw]
                        nc.vector.scalar_tensor_tensor(
                            osb, ps, b2[:, m:m + 1], xT[:, m, ts],
                            ALU.add, ALU.add,
                        )
                        nc.sync.dma_start(out=out_d[:, m, ts], in_=osb)

    nc.compile()
    return nc


def _program():
    global _NC
    if _NC is None:
        _NC = _build()
    return _NC


def host_inputs(x, w_qkv, b_qkv, w_proj, b_proj, ln1_g, ln1_b, ln2_g, ln2_b,
                w_fc1, b_fc1, w_fc2, b_fc2):
    """Fold LN gains into weights and build the per-core input maps."""
    bf = ml_dtypes.bfloat16
    x = np.asarray(x, dtype=np.float32)
    B = x.shape[0]

    # feature-major x, chunked: [B, P, KC, NTOK]
    xTt = np.ascontiguousarray(
        x.transpose(0, 2, 1).reshape(B, KC, P, NTOK).transpose(0, 2, 1, 3)
    )

    wqk_f = w_qkv[:2 * C] * ln1_g[None, :]
    wv_f = w_qkv[2 * C:] * ln1_g[None, :]
    w1_f = w_fc1 * ln2_g[None, :]

    wqkT = wqk_f.T.reshape(KC, P, 2 * KC, P).transpose(2, 1, 0, 3)
    wqkT = np.ascontiguousarray(wqkT).astype(bf)
    wvT = np.ascontiguousarray(
        wv_f.T.reshape(KC, P, C).transpose(1, 0, 2)).astype(bf)
    wpT = w_proj.T.reshape(KC, P, KC, P).transpose(2, 1, 0, 3)
    wpT = np.ascontiguousarray(wpT).astype(bf)
    w1T = w1_f.T.reshape(KC, P, MH, P).transpose(2, 1, 0, 3)
    w1T = np.ascontiguousarray(w1T).astype(bf)
    w2T = w_fc2.T.reshape(MH, P, KC, P).transpose(2, 1, 0, 3)
    w2T = np.ascontiguousarray(w2T).astype(bf)

    def pad_rows(u, b, shape):
        # row 0 = u, row 32 = b, zeros elsewhere (full-128-row correction
        # operands keep the tensor engine's weight-load pull-ahead alive)
        out = np.zeros((P,) + shape[1:], dtype=np.float32)
        out[0] = u.reshape(shape[1:])
        out[32] = b.reshape(shape[1:])
        return np.ascontiguousarray(out).astype(bf)

    uqk = -wqk_f.sum(axis=1)
    bqk = b_qkv[:2 * C] + w_qkv[:2 * C] @ ln1_b
    ubqk = pad_rows(uqk, bqk, (P, 2 * KC, P))
    uv = -wv_f.sum(axis=1)
    bv = b_qkv[2 * C:] + w_qkv[2 * C:] @ ln1_b
    uvbv = pad_rows(uv, bv, (P, C))
    u1 = -w1_f.sum(axis=1)
    b1 = b_fc1 + w_fc1 @ ln2_b
    ub1 = pad_rows(u1, b1, (P, MH, P))

    pb = np.ascontiguousarray(b_proj.reshape(KC, P).T).astype(np.float32)
    b2a = np.ascontiguousarray(b_fc2.reshape(KC, P).T).astype(np.float32)

    shared = dict(
        wqk=wqkT, wv=wvT, wp=wpT, w1=w1T, w2=w2T,
        ubqk=ubqk, uvbv=uvbv, ub1=ub1, pb=pb, b2=b2a,
    )
    in_maps = []
    for core in range(N_CORES):
        xc = np.concatenate([xTt[2 * core], xTt[2 * core + 1]], axis=2)
        in_maps.append(dict(xT=np.ascontiguousarray(xc), **shared))
    return in_maps


def unshard(results, B):
    out = np.empty((B, NTOK, C), dtype=np.float32)
    for core in range(N_CORES):
        o = results[core]["outT"]  # [P, KC, T]
        full = o.transpose(1, 0, 2).reshape(C, T)
        out[2 * core] = full[:, :NTOK].T
        out[2 * core + 1] = full[:, NTOK:].T
    return out


def kernel(x, w_qkv, b_qkv, w_proj, b_proj, ln1_g, ln1_b, ln2_g, ln2_b,
           w_fc1, b_fc1, w_fc2, b_fc2, _trace=False, _tmpdir=None):
    global LAST_EXEC_NS
    in_maps = host_inputs(
        x, w_qkv, b_qkv, w_proj, b_proj, ln1_g, ln1_b, ln2_g, ln2_b,
        w_fc1, b_fc1, w_fc2, b_fc2,
    )
    nc = _program()
    res = run_bass_kernel_spmd(
        nc, in_maps, list(range(N_CORES)), trace=_trace, tmpdir=_tmpdir
    )
    LAST_EXEC_NS = res.exec_time_ns
    return unshard(res.results, np.asarray(x).shape[0])


# revision 37
# speedup vs baseline: 1.0132x; 1.0132x over previous
"""Trainium2 Bass kernel for a ViT-style dense transformer block (v2).

Reference computation:
    xn = LN1(x); qkv = xn @ Wqkv.T + b; 16-head softmax attention;
    x = x + attn_out @ Wp.T + bp;
    out = x + gelu(LN2(x) @ W1.T + b1) @ W2.T + b2
Shapes: x [16, 577, 1024], heads 16, head_dim 64, hidden 4096.

Sharding: data-parallel over batch across 8 NeuronCores; each core gets 2
batch items concatenated along tokens (T = 2*577 = 1154). No collectives.

Structure (vs the 743us v0 baseline; measured ~606-626us):
  * LayerNorm is restructured so the device only computes
    z = x * rsqrt(var+eps) (cheap bf16 2x-mode DVE ops). gamma/beta are
    folded into the consumer weights on the host; the -mean*r*u + bias
    correction is one extra matmul accumulated into each consumer's PSUM
    group. Its operands are padded to full 128 rows (row 0 = mean*r,
    row 32 = ones, zeros elsewhere) because K=2 operands stall the
    tensor engine's LDWEIGHTS pull-ahead at every PSUM-group boundary
    (measured +200ns per group; padded operands run at the 163ns/MM
    ideal).
  * The QKV dense matmuls are emitted as small quanta interleaved into
    the attention pair loop, so the tensor engine always has dense work
    while the scalar engine streams the softmax exps. This keeps the PE
    HAM clock at 8/8 (the v0 kernel spent 226us throttled at K=4/8).
  * One-pair lookahead: scores+exps for pair p+1 are emitted before
    finish_group(p) so the exp stream never drains; exps for both heads
    of a pair are evaluated by a single ACT instruction over a 4-bank
    score tile.  q/k are stored fp8e4 (softmax averaging washes the
    quantization out of y); full fp8 matmuls were evaluated and rejected
    (e4m3 rms quantization ~3.6% per operand would blow the 2e-2 gate).
  * LN1 runs in four fine front tiles interleaved with pair-0 q/k, v,
    and even pair-0 scores; LN2 tiles 1-2 are deferred underneath fc1's
    tile-0 pass.  Weight streams issue from the otherwise-idle GPSIMD
    queue to avoid sync-queue head-of-line blocking.
  * Softmax denominators use reciprocal_approx_fast (DVE custom op);
    normalization multiplies stashed bf16 y_raw by a rank-1 PE broadcast
    of the reciprocal row.
"""

import math
import numpy as np
import ml_dtypes
from collections import deque
from contextlib import ExitStack

import concourse.bacc as bacc
import concourse.mybir as mybir
import concourse.tile as tile
from concourse.bass_utils import run_bass_kernel_spmd

F32 = mybir.dt.float32
BF16 = mybir.dt.bfloat16
FP8 = mybir.dt.float8e4
AF = mybir.ActivationFunctionType
ALU = mybir.AluOpType

N_CORES = 8
P = 128
C = 1024
KC = C // P          # 8 feature chunks
H = 16               # heads
D = 64               # head dim
HID = 4096
MH = HID // P        # 32 hidden chunks
NTOK = 577           # tokens per item
T = 2 * NTOK         # tokens per core
EPS = 1e-5

# token tiles for dense (non-attention) phases; uniform ~385 keeps every
# matmul free-dim large enough to hide LDWEIGHTS
TT = [(0, 385), (385, 385), (770, 384)]
# query tiles per item (relative to item start)
QT = [(0, 512), (512, 65)]
# key chunks per item: 4 full 128s + one 65
KCH = [(0, 128), (128, 128), (256, 128), (384, 128), (512, 65)]

LAST_EXEC_NS = None
_NC = None


def _build(num_devices=N_CORES):
    nc = bacc.Bacc(
        "TRN2", target_bir_lowering=False, debug=False, num_devices=num_devices
    )

    xT_d = nc.dram_tensor("xT", [P, KC, T], F32, kind="ExternalInput")
    wqk_d = nc.dram_tensor("wqk", [2 * KC, P, KC, P], BF16, kind="ExternalInput")
    wv_d = nc.dram_tensor("wv", [P, KC, C], BF16, kind="ExternalInput")
    wp_d = nc.dram_tensor("wp", [KC, P, KC, P], BF16, kind="ExternalInput")
    w1_d = nc.dram_tensor("w1", [MH, P, KC, P], BF16, kind="ExternalInput")
    w2_d = nc.dram_tensor("w2", [KC, P, MH, P], BF16, kind="ExternalInput")
    ubqk_d = nc.dram_tensor("ubqk", [P, 2 * KC, P], BF16, kind="ExternalInput")
    uvbv_d = nc.dram_tensor("uvbv", [P, C], BF16, kind="ExternalInput")
    ub1_d = nc.dram_tensor("ub1", [P, MH, P], BF16, kind="ExternalInput")
    pb_d = nc.dram_tensor("pb", [P, KC], F32, kind="ExternalInput")
    b2_d = nc.dram_tensor("b2", [P, KC], F32, kind="ExternalInput")
    out_d = nc.dram_tensor("outT", [P, KC, T], F32, kind="ExternalOutput")

    with tile.TileContext(nc) as tc, ExitStack() as top:
        const_p = top.enter_context(tc.tile_pool(name="consts", bufs=1))
        xp = top.enter_context(tc.tile_pool(name="xres", bufs=1))
        zp = top.enter_context(tc.tile_pool(name="zpool", bufs=1))

        pb = const_p.tile([P, KC], F32)
        nc.sync.dma_start(out=pb, in_=pb_d[:])
        b2 = const_p.tile([P, KC], F32)
        nc.sync.dma_start(out=b2, in_=b2_d[:])
        ubqk = const_p.tile([P, 2 * KC, P], BF16)
        nc.sync.dma_start(out=ubqk, in_=ubqk_d[:])
        uvbv = const_p.tile([P, C], BF16)
        nc.sync.dma_start(out=uvbv, in_=uvbv_d[:])
        inv_ones = const_p.tile([P, 1], BF16)
        nc.vector.memset(inv_ones, 1.0 / C)
        ones_pp = const_p.tile([P, P], BF16)  # ones; rank-1 lhsT at any base
        nc.vector.memset(ones_pp, 1.0)
        ones_r1 = ones_pp[0:1, :]

        # residual stream, feature-major f32 (updated in place with x2)
        xT = xp.tile([P, KC, T], F32)
        for t0, tw in ((0, 192), (192, 193), (385, 385), (770, 384)):
            nc.sync.dma_start(
                out=xT[:, :, t0:t0 + tw], in_=xT_d[:, :, t0:t0 + tw]
            )

        # z = x * rsqrt(var+eps) in bf16; rows2 = [mean*r ; ones] for the
        # rank-1 LN correction matmuls. z is reused for LN2's output too.
        z = zp.tile([P, KC, T], BF16)
        # padded correction rows: row 0 = mean*r (per tile), row 32 = ones,
        # the rest stay zero -- full-128-row lhsT/rhs keeps LDWEIGHTS
        # pull-ahead working at PSUM-group boundaries (K=2 operands stall it)
        rows2a = zp.tile([P, T], BF16)
        rows2b = zp.tile([P, T], BF16)
        nc.vector.memset(rows2a, 0.0)
        nc.vector.memset(rows2b, 0.0)
        nc.vector.memset(rows2a[32:33, :], 1.0)
        nc.vector.memset(rows2b[32:33, :], 1.0)

        def ln_tile(src, rows2, t0, tw, ln_sb, ln_ps, rb_ps=None, rb_tag="lnrb"):
            if rb_ps is None:
                rb_ps = ln_ps
            """One token tile of cheap LN: z[:, :, ts] = src*rsqrt(var+eps),
            rows2[0, ts] = mean*r."""
            ts = slice(t0, t0 + tw)
            for ci in range(KC):
                nc.scalar.copy(z[:, ci, ts], src[:, ci, ts])
            s1 = ln_ps.tile([1, 512], F32, tag="s1", name="s1", bufs=1)[:, :tw]
            s2 = ln_ps.tile([1, 512], F32, tag="s2", name="s2", bufs=1)[:, :tw]
            for ci in range(KC):
                xsq = ln_sb.tile([P, 512], BF16, tag="xsq", name="xsq", bufs=2)[:, :tw]
                nc.vector.tensor_mul(xsq, z[:, ci, ts], z[:, ci, ts])
                nc.tensor.matmul(
                    s1, inv_ones, z[:, ci, ts], start=(ci == 0), stop=(ci == KC - 1)
                )
                nc.tensor.matmul(
                    s2, inv_ones, xsq, start=(ci == 0), stop=(ci == KC - 1)
                )
            msq = ln_sb.tile([1, 512], F32, tag="msq", name="msq", bufs=1)[:, :tw]
            nc.scalar.square(msq, s1)
            veps = ln_sb.tile([1, 512], F32, tag="veps", name="veps", bufs=1)[:, :tw]
            nc.vector.scalar_tensor_tensor(veps, s2, EPS, msq, ALU.add, ALU.subtract)
            rinv = ln_sb.tile([1, 512], F32, tag="rinv", name="rinv", bufs=1)[:, :tw]
            nc.vector.reciprocal_approx_fast(rinv, veps)
            rbf = ln_sb.tile([1, 512], BF16, tag="rbf", name="rbf", bufs=1)[:, :tw]
            nc.scalar.activation(rbf, rinv, AF.Sqrt)
            nc.vector.tensor_mul(rows2[0:1, ts], s1, rbf)
            ps_rb = rb_ps.tile([P, 512], F32, tag=rb_tag, name="lnrb", bufs=2)[:, :tw]
            nc.tensor.matmul(ps_rb, ones_r1, rbf, start=True, stop=True)
            rb = ln_sb.tile([P, 512], BF16, tag="rb", name="rb", bufs=2)[:, :tw]
            nc.scalar.copy(rb, ps_rb)
            for ci in range(KC):
                nc.vector.tensor_mul(z[:, ci, ts], z[:, ci, ts], rb)

        # ---------------- attention + interleaved QKV ----------------
        attn_scope = top.enter_context(ExitStack())
        yp = attn_scope.enter_context(tc.tile_pool(name="ypool", bufs=1))
        yT = yp.tile([P, KC, T], BF16)  # attention output (pre-proj)

        # everything else attention-internal closes before proj
        inner = ExitStack()
        az = inner.enter_context(tc.tile_pool(name="azpool", bufs=1))
        wv_pool = inner.enter_context(tc.tile_pool(name="wvp", bufs=1))
        wqk_pool = inner.enter_context(tc.tile_pool(name="wqkp", bufs=3))
        at_sb = inner.enter_context(tc.tile_pool(name="atsb", bufs=2))

        # q/k live in fp8e4: the softmax averages ~500 weights, so per-score
        # quantization noise washes out of y; halves the qkT footprint
        qkT = az.tile([P, 2 * KC, T], FP8)
        v_aug = az.tile([P, 10, H, 65], BF16)
        nc.vector.memset(v_aug[:, :, :, 64:65], 1.0)

        wv = wv_pool.tile([P, KC, C], BF16)
        nc.gpsimd.dma_start(out=wv, in_=wv_d[:])

        s_attn = ExitStack()
        mm_ps = s_attn.enter_context(
            tc.tile_pool(name="mmps", bufs=2, space="PSUM")
        )

        def emit_qk_tile(m, wqkb, t0, tw):
            ts = slice(t0, t0 + tw)
            ps = mm_ps.tile([P, 512], F32, tag="mm", name="qk", bufs=2)[:, :tw]
            nc.tensor.matmul(
                ps, ubqk[:, m, :], rows2a[:, ts], start=True, stop=False
            )
            for ci in range(KC):
                nc.tensor.matmul(
                    ps, wqkb[:, ci, :], z[:, ci, ts],
                    start=False, stop=(ci == KC - 1),
                )
            nc.scalar.copy(qkT[:, m, ts], ps)

        def qk_quanta(m):
            state = {}

            def mk(t_idx):
                def run():
                    if "w" not in state:
                        w = wqk_pool.tile([P, KC, P], BF16, tag="wqkb", name="wqkb")
                        nc.gpsimd.dma_start(out=w, in_=wqk_d[m])
                        state["w"] = w
                    emit_qk_tile(m, state["w"], *TT[t_idx])
                return run

            return [mk(i) for i in range(3)]

        def v_quantum(it, half, c_i):
            def run():
                k0, kw = KCH[c_i]
                tok0 = it * NTOK + k0
                tks = slice(tok0, tok0 + kw)
                hs = slice(half * 512, (half + 1) * 512)
                ps = mm_ps.tile([P, 512], F32, tag="mm", name="v", bufs=2)
                nc.tensor.matmul(
                    ps[:kw, :], rows2a[:, tks], uvbv[:, hs],
                    start=True, stop=False,
                )
                for ci in range(KC):
                    nc.tensor.matmul(
                        ps[:kw, :], z[:, ci, tks], wv[:, ci, hs],
                        start=False, stop=(ci == KC - 1),
                    )
                dst = v_aug[:kw, it * 5 + c_i, half * 8:(half + 1) * 8, 0:64]
                nc.vector.tensor_copy(
                    out=dst, in_=ps[:kw].rearrange("p (h d) -> p h d", d=64)
                )
            return run


        # dense quanta fed into the attention pair loop.  need = checkpoint
        # index by which the quantum must have been emitted (2p = before
        # pair p scores, 2p+1 = before pair p attn@v).
        quanta = deque()
        for p, need in ((1, 2), (2, 4), (3, 6)):
            for q in qk_quanta(p) + qk_quanta(KC + p):
                quanta.append((need, q))
        for c_i in range(5):
            quanta.append((8, v_quantum(0, 1, c_i)))
        for q in qk_quanta(4) + qk_quanta(KC + 4):
            quanta.append((8, q))
        for c_i in range(5):
            quanta.append((9, v_quantum(1, 1, c_i)))
        for p, need in ((5, 10), (6, 12), (7, 14)):
            for q in qk_quanta(p) + qk_quanta(KC + p):
                quanta.append((need, q))

        budget = [0]
        cur = [0]

        def drain(k):
            while quanta and quanta[0][0] <= k:
                quanta.popleft()[1]()

        def feed():
            # hold back quanta needed >2 pairs out so the tail pairs keep
            # dense work; mid-loop pairs are PE-bound anyway
            if budget[0] > 0 and quanta and quanta[0][0] <= 2 * cur[0] + 4:
                quanta.popleft()[1]()
                budget[0] -= 1

        def emit_scores(it, pair):
            """Scores + exp for both heads of a pair; one exp instruction per
            key chunk covers both heads ([kw, 2, 577] strided over the 4-bank
            score tile)."""
            exps = at_sb.tile(
                [P, 2, 5, NTOK], BF16, tag="exps", name="exps", bufs=3
            )
            for c_i, (k0, kw) in enumerate(KCH):
                ks = slice(it * NTOK + k0, it * NTOK + k0 + kw)
                # hh stride of 1024 fp32 = 2 banks keeps every matmul output
                # inside a single PSUM bank
                ps_s = mm_ps.tile(
                    [P, 2, 1024], F32, tag="score", name="score", bufs=1
                )
                for hh in range(2):
                    rows = slice(hh * D, (hh + 1) * D)
                    for q0, qw in QT:
                        qs = slice(it * NTOK + q0, it * NTOK + q0 + qw)
                        nc.tensor.matmul(
                            ps_s[:kw, hh, q0:q0 + qw],
                            qkT[rows, KC + pair, ks],
                            qkT[rows, pair, qs],
                            start=True, stop=True,
                        )
                nc.scalar.activation(
                    exps[:kw, :, c_i, :], ps_s[:kw, :, :NTOK],
                    AF.Exp, scale=0.125,
                )
                feed()
            return exps


        def emit_attnv(it, pair, exps, coll):
            yraws = []
            for q0, qw in QT:
                for hh in range(2):
                    h = 2 * pair + hh
                    base = 32 * (2 * it + hh)
                    ps_y = at_ps.tile(
                        [P, 512], F32, tag="y", name="y", bufs=2
                    )[:65, :qw]
                    for c_i, (k0, kw) in enumerate(KCH):
                        nc.tensor.matmul(
                            ps_y,
                            v_aug[:kw, it * 5 + c_i, h, :],
                            exps[:kw, hh, c_i, q0:q0 + qw],
                            start=(c_i == 0), stop=(c_i == 4),
                        )
                    nc.vector.tensor_copy(
                        out=coll[base:base + 1, q0:q0 + qw], in_=ps_y[64:65, :]
                    )
                    if q0 == 0:
                        yr = at_sb.tile(
                            [D, NTOK], BF16, tag="yraw", name="yraw", bufs=4
                        )
                        yraws.append(yr)
                    else:
                        yr = yraws[hh]
                    nc.vector.tensor_copy(out=yr[:, q0:q0 + qw], in_=ps_y[0:D, :])
                    feed()
            return yraws

        def finish_group(pair, coll, yraws2):
            recf = at_sb.tile([97, NTOK], F32, tag="recf", name="recf", bufs=1)
            nc.vector.reciprocal_approx_fast(recf, coll)
            recbf = at_sb.tile([97, NTOK], BF16, tag="recbf", name="recbf", bufs=1)
            # DVE, not ACT: keeps the exp stream free of this dependency
            nc.vector.tensor_copy(out=recbf, in_=recf)
            for it in range(2):
                for hh in range(2):
                    base = 32 * (2 * it + hh)
                    for q0, qw in QT:
                        qs = slice(it * NTOK + q0, it * NTOK + q0 + qw)
                        ps_rb = at_ps.tile(
                            [P, 512], F32, tag="y", name="rb", bufs=2
                        )[:D, :qw]
                        nc.tensor.matmul(
                            ps_rb,
                            ones_pp[base:base + 1, 0:D],
                            recbf[base:base + 1, q0:q0 + qw],
                            start=True, stop=True,
                            tile_position=(base, 0),
                        )
                        nc.vector.tensor_tensor(
                            out=yT[hh * D:(hh + 1) * D, pair, qs],
                            in0=yraws2[it][hh][:, q0:q0 + qw],
                            in1=ps_rb, op=ALU.mult,
                        )

        # front: LN1 tiles interleaved with pair-0 q/k, v(half0) and even
        # pair-0 scores, so both PE and ACT ramp as early as possible
        wqk0 = {}
        for m in (0, KC):
            wqk0[m] = wqk_pool.tile([P, KC, P], BF16, tag="wqkb", name="wqkb")
            nc.gpsimd.dma_start(out=wqk0[m], in_=wqk_d[m])
        FTT = [(0, 192), (192, 193), (385, 385), (770, 384)]
        front_v = {0: [(0, 0, 0)],
                   1: [(0, 0, 1), (0, 0, 2)],
                   2: [(0, 0, 3), (0, 0, 4), (1, 0, 0)],
                   3: [(1, 0, 1), (1, 0, 2), (1, 0, 3), (1, 0, 4)]}
        exps_cur = [None, None]
        with ExitStack() as lnx:
            ln_sb = lnx.enter_context(tc.tile_pool(name="ln1sb", bufs=2))
            ln_ps = lnx.enter_context(
                tc.tile_pool(name="ln1ps", bufs=1, space="PSUM")
            )
            for t_idx, (t0, tw) in enumerate(FTT):
                ln_tile(xT, rows2a, t0, tw, ln_sb, ln_ps,
                        rb_ps=mm_ps, rb_tag="mm")
                for m in (0, KC):
                    emit_qk_tile(m, wqk0[m], t0, tw)
                for it, half, c_i in front_v[t_idx]:
                    v_quantum(it, half, c_i)()
                if t_idx == 2:
                    # item0 spans tokens 0..577 < 770, so its pair-0
                    # scores only need z/qkT through token 770
                    exps_cur[0] = emit_scores(0, 0)
            exps_cur[1] = emit_scores(1, 0)

        at_ps = s_attn.enter_context(
            tc.tile_pool(name="atps", bufs=1, space="PSUM")
        )
        wp_early = inner.enter_context(tc.tile_pool(name="wpe", bufs=1))
        wpb_early = []
        for m2 in (0, 1):
            wt = wp_early.tile([P, KC, P], BF16, tag=f"wpe{m2}", name="wpe")
            nc.sync.dma_start(out=wt, in_=wp_d[m2])
            wpb_early.append(wt)
        early_pj = []

        # one-pair lookahead: scores+exps for pair p+1 are emitted before
        # finish_group(p), so the scalar engine's exp stream never drains
        budget[0] = max(1, math.ceil(len(quanta) / 8))
        # one persistent coll: every writer/reader is on the in-order DVE
        # queue, so no per-pair buffer rotation or re-memset is needed
        coll = at_sb.tile([97, NTOK], F32, tag="coll", name="coll", bufs=1)
        nc.vector.memset(coll, 1.0)
        for pair in range(H // 2):
            drain(2 * pair + 1)
            yr0 = emit_attnv(0, pair, exps_cur[0], coll)
            yr1 = emit_attnv(1, pair, exps_cur[1], coll)
            if pair < H // 2 - 1:
                drain(2 * pair + 2)
                cur[0] = pair + 1
                budget[0] = max(1, math.ceil(len(quanta) / (7 - pair)))
                exps_cur = [emit_scores(0, pair + 1), emit_scores(1, pair + 1)]
            else:
                while quanta:
                    quanta.popleft()[1]()
                # partial proj groups for (tile0, m=0,1) fill the last
                # pair's exp-wait stalls; ci=7 lands after finish_group(7)
                t0e, twe = TT[0]
                tse = slice(t0e, t0e + twe)
                for m2 in (0, 1):
                    ps = mm_ps.tile(
                        [P, 512], F32, tag="mm", name="pj0", bufs=2
                    )[:, :twe]
                    early_pj.append(ps)
                    for ci in range(KC - 1):
                        nc.tensor.matmul(
                            ps, wpb_early[m2][:, ci, :], yT[:, ci, tse],
                            start=(ci == 0), stop=False,
                        )
            finish_group(pair, coll, [yr0, yr1])
        t0e, twe = TT[0]
        tse = slice(t0e, t0e + twe)
        for m2 in (0, 1):
            nc.tensor.matmul(
                early_pj[m2], wpb_early[m2][:, KC - 1, :], yT[:, KC - 1, tse],
                start=False, stop=True,
            )
            nc.vector.scalar_tensor_tensor(
                xT[:, m2, tse], early_pj[m2], pb[:, m2:m2 + 1], xT[:, m2, tse],
                ALU.add, ALU.add,
            )

        s_attn.close()
        inner.close()  # free qkT / v_aug / exps / wv / wqk before proj

        # ---------------- proj + residual + LN2 (per tile) ----------------
        w2_pool = top.enter_context(tc.tile_pool(name="w2pool", bufs=3))
        w2bs = {}
        for m in (0, 1):
            w2bs[m] = w2_pool.tile([P, MH, P], BF16, tag="w2b", name="w2b")
            nc.sync.dma_start(out=w2bs[m], in_=w2_d[m])
        ln2_sb = top.enter_context(tc.tile_pool(name="ln2sb", bufs=2))
        ln2_ps = top.enter_context(
            tc.tile_pool(name="ln2ps", bufs=1, space="PSUM")
        )
        with ExitStack() as pjx:
            pj_ps = pjx.enter_context(
                tc.tile_pool(name="pjps", bufs=2, space="PSUM")
            )
            wp_pool = pjx.enter_context(tc.tile_pool(name="wpp", bufs=3))
            for t0, tw in TT:
                ts = slice(t0, t0 + tw)
                for m in range(KC):
                    if t0 == 0 and m < 2:
                        continue  # done early, at the attention tail
                    wpb = wp_pool.tile([P, KC, P], BF16, tag="wpb", name="wpb")
                    nc.gpsimd.dma_start(out=wpb, in_=wp_d[m])
                    ps = pj_ps.tile([P, 512], F32, tag="z", name="z", bufs=2)[:, :tw]
                    for ci in range(KC):
                        nc.tensor.matmul(
                            ps, wpb[:, ci, :], yT[:, ci, ts],
                            start=(ci == 0), stop=(ci == KC - 1),
                        )
                    # x2 = (z + bp) + x   (in place into xT)
                    nc.vector.scalar_tensor_tensor(
                        xT[:, m, ts], ps, pb[:, m:m + 1], xT[:, m, ts],
                        ALU.add, ALU.add,
                    )
                if t0 == 0:
                    # only tile0's LN2 gates fc1-A; the rest run under it
                    ln_tile(xT, rows2b, t0, tw, ln2_sb, ln2_ps)


        # ------------------------- MLP -------------------------
        with ExitStack() as mx:
            mlpc = mx.enter_context(tc.tile_pool(name="mlpc", bufs=1))
            h_p = mx.enter_context(tc.tile_pool(name="hpool", bufs=1))
            ub1 = mlpc.tile([P, MH, P], BF16)
            nc.gpsimd.dma_start(out=ub1, in_=ub1_d[:])
            hT = h_p.tile([P, MH, T], BF16)

            # fc1 in two tile groups so it can start right after LN2(t0);
            # one shared weight pool with prefetch across the A/B boundary
            with ExitStack() as f1x:
                w1_pool = f1x.enter_context(tc.tile_pool(name="w1pool", bufs=4))
                f1_ps = f1x.enter_context(
                    tc.tile_pool(name="f1ps", bufs=4, space="PSUM")
                )
                steps = [(TT[:1], mh) for mh in range(MH)]
                steps += [(TT[1:], mh) for mh in range(MH)]
                w1bs = {}

                def fetch_w1(i):
                    if i < len(steps) and i not in w1bs:
                        w1bs[i] = w1_pool.tile(
                            [P, KC, P], BF16, tag="w1b", name="w1b"
                        )
                        nc.gpsimd.dma_start(out=w1bs[i], in_=w1_d[steps[i][1]])

                for i, (tgroup, mh) in enumerate(steps):
                    if i == 4:
                        # deferred LN2 tiles run underneath fc1-A
                        ln_tile(xT, rows2b, *TT[1], ln2_sb, ln2_ps)
                    elif i == 8:
                        ln_tile(xT, rows2b, *TT[2], ln2_sb, ln2_ps)
                    fetch_w1(i)
                    fetch_w1(i + 1)
                    fetch_w1(i + 2)
                    w1b = w1bs.pop(i)
                    for t0, tw in tgroup:
                        ts = slice(t0, t0 + tw)
                        ps = f1_ps.tile([P, 512], F32, tag="h", name="h")[:, :tw]
                        nc.tensor.matmul(
                            ps, ub1[:, mh, :], rows2b[:, ts],
                            start=True, stop=False,
                        )
                        for ci in range(KC):
                            nc.tensor.matmul(
                                ps, w1b[:, ci, :], z[:, ci, ts],
                                start=False, stop=(ci == KC - 1),
                            )
                        nc.scalar.activation(hT[:, mh, ts], ps, AF.Gelu)

            with ExitStack() as f2x:
                f2_ps = f2x.enter_context(
                    tc.tile_pool(name="f2ps", bufs=2, space="PSUM")
                )
                o_pool = f2x.enter_context(tc.tile_pool(name="opool", bufs=3))
                for m in range(KC):
                    if m not in w2bs:
                        w2bs[m] = w2_pool.tile([P, MH, P], BF16, tag="w2b", name="w2b")
                        nc.sync.dma_start(out=w2bs[m], in_=w2_d[m])
                    if m + 1 < KC:
                        # prefetch next block so its 1MB DMA hides under MMs
                        w2bs[m + 1] = w2_pool.tile(
                            [P, MH, P], BF16, tag="w2b", name="w2b"
                        )
                        nc.sync.dma_start(out=w2bs[m + 1], in_=w2_d[m + 1])
                    w2b = w2bs[m]
                    for t0, tw in TT:
                        ts = slice(t0, t0 + tw)
                        ps = f2_ps.tile([P, 512], F32, tag="o", name="o")[:, :tw]
                        for kh in range(MH):
                            nc.tensor.matmul(
                                ps, w2b[:, kh, :], hT[:, kh, ts],
                                start=(kh == 0), stop=(kh == MH - 1),
                            )
                        osb = o_pool.tile([P, 512], F32, tag="osb", name="osb")[:, :tw]
                        nc.vector.scalar_tensor_tensor(
                            osb, ps, b2[:, m:m + 1], xT[:, m, ts],
                            ALU.add, ALU.add,
                        )
                        nc.sync.dma_start(out=out_d[:, m, ts], in_=osb)

    nc.compile()
    return nc


def _program():
    global _NC
    if _NC is None:
        _NC = _build()
    return _NC


def host_inputs(x, w_qkv, b_qkv, w_proj, b_proj, ln1_g, ln1_b, ln2_g, ln2_b,
                w_fc1, b_fc1, w_fc2, b_fc2):
    """Fold LN gains into weights and build the per-core input maps."""
    bf = ml_dtypes.bfloat16
    x = np.asarray(x, dtype=np.float32)
    B = x.shape[0]

    # feature-major x, chunked: [B, P, KC, NTOK]
    xTt = np.ascontiguousarray(
        x.transpose(0, 2, 1).reshape(B, KC, P, NTOK).transpose(0, 2, 1, 3)
    )

    wqk_f = w_qkv[:2 * C] * ln1_g[None, :]
    wv_f = w_qkv[2 * C:] * ln1_g[None, :]
    w1_f = w_fc1 * ln2_g[None, :]

    wqkT = wqk_f.T.reshape(KC, P, 2 * KC, P).transpose(2, 1, 0, 3)
    wqkT = np.ascontiguousarray(wqkT).astype(bf)
    wvT = np.ascontiguousarray(
        wv_f.T.reshape(KC, P, C).transpose(1, 0, 2)).astype(bf)
    wpT = w_proj.T.reshape(KC, P, KC, P).transpose(2, 1, 0, 3)
    wpT = np.ascontiguousarray(wpT).astype(bf)
    w1T = w1_f.T.reshape(KC, P, MH, P).transpose(2, 1, 0, 3)
    w1T = np.ascontiguousarray(w1T).astype(bf)
    w2T = w_fc2.T.reshape(MH, P, KC, P).transpose(2, 1, 0, 3)
    w2T = np.ascontiguousarray(w2T).astype(bf)

    def pad_rows(u, b, shape):
        # row 0 = u, row 32 = b, zeros elsewhere (full-128-row correction
        # operands keep the tensor engine's weight-load pull-ahead alive)
        out = np.zeros((P,) + shape[1:], dtype=np.float32)
        out[0] = u.reshape(shape[1:])
        out[32] = b.reshape(shape[1:])
        return np.ascontiguousarray(out).astype(bf)

    uqk = -wqk_f.sum(axis=1)
    bqk = b_qkv[:2 * C] + w_qkv[:2 * C] @ ln1_b
    ubqk = pad_rows(uqk, bqk, (P, 2 * KC, P))
    uv = -wv_f.sum(axis=1)
    bv = b_qkv[2 * C:] + w_qkv[2 * C:] @ ln1_b
    uvbv = pad_rows(uv, bv, (P, C))
    u1 = -w1_f.sum(axis=1)
    b1 = b_fc1 + w_fc1 @ ln2_b
    ub1 = pad_rows(u1, b1, (P, MH, P))

    pb = np.ascontiguousarray(b_proj.reshape(KC, P).T).astype(np.float32)
    b2a = np.ascontiguousarray(b_fc2.reshape(KC, P).T).astype(np.float32)

    shared = dict(
        wqk=wqkT, wv=wvT, wp=wpT, w1=w1T, w2=w2T,
        ubqk=ubqk, uvbv=uvbv, ub1=ub1, pb=pb, b2=b2a,
    )
    in_maps = []
    for core in range(N_CORES):
        xc = np.concatenate([xTt[2 * core], xTt[2 * core + 1]], axis=2)
        in_maps.append(dict(xT=np.ascontiguousarray(xc), **shared))
    return in_maps


def unshard(results, B):
    out = np.empty((B, NTOK, C), dtype=np.float32)
    for core in range(N_CORES):
        o = results[core]["outT"]  # [P, KC, T]
        full = o.transpose(1, 0, 2).reshape(C, T)
        out[2 * core] = full[:, :NTOK].T
        out[2 * core + 1] = full[:, NTOK:].T
    return out


def kernel(x, w_qkv, b_qkv, w_proj, b_proj, ln1_g, ln1_b, ln2_g, ln2_b,
           w_fc1, b_fc1, w_fc2, b_fc2, _trace=False, _tmpdir=None):
    global LAST_EXEC_NS
    in_maps = host_inputs(
        x, w_qkv, b_qkv, w_proj, b_proj, ln1_g, ln1_b, ln2_g, ln2_b,
        w_fc1, b_fc1, w_fc2, b_fc2,
    )
    nc = _program()
    res = run_bass_kernel_spmd(
        nc, in_maps, list(range(N_CORES)), trace=_trace, tmpdir=_tmpdir
    )
    LAST_EXEC_NS = res.exec_time_ns
    return unshard(res.results, np.asarray(x).shape[0])


# revision 38
# speedup vs baseline: 1.0159x; 1.0027x over previous
"""Trainium2 Bass kernel for a ViT-style dense transformer block (v2).

Reference computation:
    xn = LN1(x); qkv = xn @ Wqkv.T + b; 16-head softmax attention;
    x = x + attn_out @ Wp.T + bp;
    out = x + gelu(LN2(x) @ W1.T + b1) @ W2.T + b2
Shapes: x [16, 577, 1024], heads 16, head_dim 64, hidden 4096.

Sharding: data-parallel over batch across 8 NeuronCores; each core gets 2
batch items concatenated along tokens (T = 2*577 = 1154). No collectives.

Structure (vs the 743us v0 baseline; measured ~606-626us):
  * LayerNorm is restructured so the device only computes
    z = x * rsqrt(var+eps) (cheap bf16 2x-mode DVE ops). gamma/beta are
    folded into the consumer weights on the host; the -mean*r*u + bias
    correction is one extra matmul accumulated into each consumer's PSUM
    group. Its operands are padded to full 128 rows (row 0 = mean*r,
    row 32 = ones, zeros elsewhere) because K=2 operands stall the
    tensor engine's LDWEIGHTS pull-ahead at every PSUM-group boundary
    (measured +200ns per group; padded operands run at the 163ns/MM
    ideal).
  * The QKV dense matmuls are emitted as small quanta interleaved into
    the attention pair loop, so the tensor engine always has dense work
    while the scalar engine streams the softmax exps. This keeps the PE
    HAM clock at 8/8 (the v0 kernel spent 226us throttled at K=4/8).
  * One-pair lookahead: scores+exps for pair p+1 are emitted before
    finish_group(p) so the exp stream never drains; exps for both heads
    of a pair are evaluated by a single ACT instruction over a 4-bank
    score tile.  q/k are stored fp8e4 (softmax averaging washes the
    quantization out of y); full fp8 matmuls were evaluated and rejected
    (e4m3 rms quantization ~3.6% per operand would blow the 2e-2 gate).
  * LN1 runs in four fine front tiles interleaved with pair-0 q/k, v,
    and even pair-0 scores; LN2 tiles 1-2 are deferred underneath fc1's
    tile-0 pass.  Weight streams issue from the otherwise-idle GPSIMD
    queue to avoid sync-queue head-of-line blocking.
  * Softmax denominators use reciprocal_approx_fast (DVE custom op);
    normalization multiplies stashed bf16 y_raw by a rank-1 PE broadcast
    of the reciprocal row.
"""

import math
import numpy as np
import ml_dtypes
from collections import deque
from contextlib import ExitStack

import concourse.bacc as bacc
import concourse.mybir as mybir
import concourse.tile as tile
from concourse.bass_utils import run_bass_kernel_spmd

F32 = mybir.dt.float32
BF16 = mybir.dt.bfloat16
FP8 = mybir.dt.float8e4
AF = mybir.ActivationFunctionType
ALU = mybir.AluOpType

N_CORES = 8
P = 128
C = 1024
KC = C // P          # 8 feature chunks
H = 16               # heads
D = 64               # head dim
HID = 4096
MH = HID // P        # 32 hidden chunks
NTOK = 577           # tokens per item
T = 2 * NTOK         # tokens per core
EPS = 1e-5

# token tiles for dense (non-attention) phases; uniform ~385 keeps every
# matmul free-dim large enough to hide LDWEIGHTS
TT = [(0, 385), (385, 385), (770, 384)]
# query tiles per item (relative to item start)
QT = [(0, 512), (512, 65)]
# key chunks per item: 4 full 128s + one 65
KCH = [(0, 128), (128, 128), (256, 128), (384, 128), (512, 65)]

LAST_EXEC_NS = None
_NC = None


def _build(num_devices=N_CORES):
    nc = bacc.Bacc(
        "TRN2", target_bir_lowering=False, debug=False, num_devices=num_devices
    )

    xT_d = nc.dram_tensor("xT", [P, KC, T], F32, kind="ExternalInput")
    wqk_d = nc.dram_tensor("wqk", [2 * KC, P, KC, P], BF16, kind="ExternalInput")
    wv_d = nc.dram_tensor("wv", [P, KC, C], BF16, kind="ExternalInput")
    wp_d = nc.dram_tensor("wp", [KC, P, KC, P], BF16, kind="ExternalInput")
    w1_d = nc.dram_tensor("w1", [MH, P, KC, P], BF16, kind="ExternalInput")
    w2_d = nc.dram_tensor("w2", [KC, P, MH, P], BF16, kind="ExternalInput")
    ubqk_d = nc.dram_tensor("ubqk", [P, 2 * KC, P], BF16, kind="ExternalInput")
    uvbv_d = nc.dram_tensor("uvbv", [P, C], BF16, kind="ExternalInput")
    ub1_d = nc.dram_tensor("ub1", [P, MH, P], BF16, kind="ExternalInput")
    pb_d = nc.dram_tensor("pb", [P, KC], F32, kind="ExternalInput")
    b2_d = nc.dram_tensor("b2", [P, KC], F32, kind="ExternalInput")
    out_d = nc.dram_tensor("outT", [P, KC, T], F32, kind="ExternalOutput")

    with tile.TileContext(nc) as tc, ExitStack() as top:
        const_p = top.enter_context(tc.tile_pool(name="consts", bufs=1))
        xp = top.enter_context(tc.tile_pool(name="xres", bufs=1))
        zp = top.enter_context(tc.tile_pool(name="zpool", bufs=1))

        pb = const_p.tile([P, KC], F32)
        nc.sync.dma_start(out=pb, in_=pb_d[:])
        b2 = const_p.tile([P, KC], F32)
        nc.sync.dma_start(out=b2, in_=b2_d[:])
        ubqk = const_p.tile([P, 2 * KC, P], BF16)
        nc.sync.dma_start(out=ubqk, in_=ubqk_d[:])
        uvbv = const_p.tile([P, C], BF16)
        nc.sync.dma_start(out=uvbv, in_=uvbv_d[:])
        inv_ones = const_p.tile([P, 1], BF16)
        nc.vector.memset(inv_ones, 1.0 / C)
        ones_pp = const_p.tile([P, P], BF16)  # ones; rank-1 lhsT at any base
        nc.vector.memset(ones_pp, 1.0)
        ones_r1 = ones_pp[0:1, :]

        # residual stream, feature-major f32 (updated in place with x2)
        xT = xp.tile([P, KC, T], F32)
        for t0, tw in ((0, 192), (192, 193), (385, 385), (770, 384)):
            nc.sync.dma_start(
                out=xT[:, :, t0:t0 + tw], in_=xT_d[:, :, t0:t0 + tw]
            )

        # z = x * rsqrt(var+eps) in bf16; rows2 = [mean*r ; ones] for the
        # rank-1 LN correction matmuls. z is reused for LN2's output too.
        z = zp.tile([P, KC, T], BF16)
        # padded correction rows: row 0 = mean*r (per tile), row 32 = ones,
        # the rest stay zero -- full-128-row lhsT/rhs keeps LDWEIGHTS
        # pull-ahead working at PSUM-group boundaries (K=2 operands stall it)
        rows2a = zp.tile([P, T], BF16)
        rows2b = zp.tile([P, T], BF16)
        nc.vector.memset(rows2a, 0.0)
        nc.vector.memset(rows2b, 0.0)
        nc.vector.memset(rows2a[32:33, :], 1.0)
        nc.vector.memset(rows2b[32:33, :], 1.0)

        def ln_tile(src, rows2, t0, tw, ln_sb, ln_ps, rb_ps=None, rb_tag="lnrb"):
            if rb_ps is None:
                rb_ps = ln_ps
            """One token tile of cheap LN: z[:, :, ts] = src*rsqrt(var+eps),
            rows2[0, ts] = mean*r."""
            ts = slice(t0, t0 + tw)
            for ci in range(KC):
                nc.scalar.copy(z[:, ci, ts], src[:, ci, ts])
            s1 = ln_ps.tile([1, 512], F32, tag="s1", name="s1", bufs=1)[:, :tw]
            s2 = ln_ps.tile([1, 512], F32, tag="s2", name="s2", bufs=1)[:, :tw]
            for ci in range(KC):
                xsq = ln_sb.tile([P, 512], BF16, tag="xsq", name="xsq", bufs=2)[:, :tw]
                nc.vector.tensor_mul(xsq, z[:, ci, ts], z[:, ci, ts])
                nc.tensor.matmul(
                    s1, inv_ones, z[:, ci, ts], start=(ci == 0), stop=(ci == KC - 1)
                )
                nc.tensor.matmul(
                    s2, inv_ones, xsq, start=(ci == 0), stop=(ci == KC - 1)
                )
            msq = ln_sb.tile([1, 512], F32, tag="msq", name="msq", bufs=1)[:, :tw]
            nc.scalar.square(msq, s1)
            veps = ln_sb.tile([1, 512], F32, tag="veps", name="veps", bufs=1)[:, :tw]
            nc.vector.scalar_tensor_tensor(veps, s2, EPS, msq, ALU.add, ALU.subtract)
            rinv = ln_sb.tile([1, 512], F32, tag="rinv", name="rinv", bufs=1)[:, :tw]
            nc.vector.reciprocal_approx_fast(rinv, veps)
            rbf = ln_sb.tile([1, 512], BF16, tag="rbf", name="rbf", bufs=1)[:, :tw]
            nc.scalar.activation(rbf, rinv, AF.Sqrt)
            nc.vector.tensor_mul(rows2[0:1, ts], s1, rbf)
            ps_rb = rb_ps.tile([P, 512], F32, tag=rb_tag, name="lnrb", bufs=2)[:, :tw]
            nc.tensor.matmul(ps_rb, ones_r1, rbf, start=True, stop=True)
            rb = ln_sb.tile([P, 512], BF16, tag="rb", name="rb", bufs=2)[:, :tw]
            nc.scalar.copy(rb, ps_rb)
            for ci in range(KC):
                nc.vector.tensor_mul(z[:, ci, ts], z[:, ci, ts], rb)

        # ---------------- attention + interleaved QKV ----------------
        attn_scope = top.enter_context(ExitStack())
        yp = attn_scope.enter_context(tc.tile_pool(name="ypool", bufs=1))
        yT = yp.tile([P, KC, T], BF16)  # attention output (pre-proj)

        # everything else attention-internal closes before proj
        inner = ExitStack()
        az = inner.enter_context(tc.tile_pool(name="azpool", bufs=1))
        wv_pool = inner.enter_context(tc.tile_pool(name="wvp", bufs=1))
        wqk_pool = inner.enter_context(tc.tile_pool(name="wqkp", bufs=3))
        at_sb = inner.enter_context(tc.tile_pool(name="atsb", bufs=2))

        # q/k live in fp8e4: the softmax averages ~500 weights, so per-score
        # quantization noise washes out of y; halves the qkT footprint
        qkT = az.tile([P, 2 * KC, T], FP8)
        v_aug = az.tile([P, 10, H, 65], BF16)
        nc.vector.memset(v_aug[:, :, :, 64:65], 1.0)

        wv = wv_pool.tile([P, KC, C], BF16)
        nc.gpsimd.dma_start(out=wv, in_=wv_d[:])

        s_attn = ExitStack()
        mm_ps = s_attn.enter_context(
            tc.tile_pool(name="mmps", bufs=2, space="PSUM")
        )

        def emit_qk_tile(m, wqkb, t0, tw):
            ts = slice(t0, t0 + tw)
            ps = mm_ps.tile([P, 512], F32, tag="mm", name="qk", bufs=2)[:, :tw]
            nc.tensor.matmul(
                ps, ubqk[:, m, :], rows2a[:, ts], start=True, stop=False
            )
            for ci in range(KC):
                nc.tensor.matmul(
                    ps, wqkb[:, ci, :], z[:, ci, ts],
                    start=False, stop=(ci == KC - 1),
                )
            nc.scalar.copy(qkT[:, m, ts], ps)

        def qk_quanta(m):
            state = {}

            def mk(t_idx):
                def run():
                    if "w" not in state:
                        w = wqk_pool.tile([P, KC, P], BF16, tag="wqkb", name="wqkb")
                        nc.gpsimd.dma_start(out=w, in_=wqk_d[m])
                        state["w"] = w
                    emit_qk_tile(m, state["w"], *TT[t_idx])
                return run

            return [mk(i) for i in range(3)]

        def v_quantum(it, half, c_i):
            def run():
                k0, kw = KCH[c_i]
                tok0 = it * NTOK + k0
                tks = slice(tok0, tok0 + kw)
                hs = slice(half * 512, (half + 1) * 512)
                ps = mm_ps.tile([P, 512], F32, tag="mm", name="v", bufs=2)
                nc.tensor.matmul(
                    ps[:kw, :], rows2a[:, tks], uvbv[:, hs],
                    start=True, stop=False,
                )
                for ci in range(KC):
                    nc.tensor.matmul(
                        ps[:kw, :], z[:, ci, tks], wv[:, ci, hs],
                        start=False, stop=(ci == KC - 1),
                    )
                dst = v_aug[:kw, it * 5 + c_i, half * 8:(half + 1) * 8, 0:64]
                nc.vector.tensor_copy(
                    out=dst, in_=ps[:kw].rearrange("p (h d) -> p h d", d=64)
                )
            return run


        # dense quanta fed into the attention pair loop.  need = checkpoint
        # index by which the quantum must have been emitted (2p = before
        # pair p scores, 2p+1 = before pair p attn@v).
        quanta = deque()
        for p, need in ((1, 2), (2, 4), (3, 6)):
            for q in qk_quanta(p) + qk_quanta(KC + p):
                quanta.append((need, q))
        for c_i in range(5):
            quanta.append((8, v_quantum(0, 1, c_i)))
        for q in qk_quanta(4) + qk_quanta(KC + 4):
            quanta.append((8, q))
        for c_i in range(5):
            quanta.append((9, v_quantum(1, 1, c_i)))
        for p, need in ((5, 10), (6, 12), (7, 14)):
            for q in qk_quanta(p) + qk_quanta(KC + p):
                quanta.append((need, q))

        budget = [0]
        cur = [0]

        def drain(k):
            while quanta and quanta[0][0] <= k:
                quanta.popleft()[1]()

        def feed():
            # hold back quanta needed >2 pairs out so the tail pairs keep
            # dense work; mid-loop pairs are PE-bound anyway
            if budget[0] > 0 and quanta and quanta[0][0] <= 2 * cur[0] + 4:
                quanta.popleft()[1]()
                budget[0] -= 1

        def emit_scores(it, pair):
            """Scores + exp for both heads of a pair; one exp instruction per
            key chunk covers both heads ([kw, 2, 577] strided over the 4-bank
            score tile)."""
            exps = at_sb.tile(
                [P, 2, 5, NTOK], BF16, tag="exps", name="exps", bufs=3
            )
            for c_i, (k0, kw) in enumerate(KCH):
                ks = slice(it * NTOK + k0, it * NTOK + k0 + kw)
                # hh stride of 1024 fp32 = 2 banks keeps every matmul output
                # inside a single PSUM bank
                ps_s = mm_ps.tile(
                    [P, 2, 1024], F32, tag="score", name="score", bufs=1
                )
                for hh in range(2):
                    rows = slice(hh * D, (hh + 1) * D)
                    for q0, qw in QT:
                        qs = slice(it * NTOK + q0, it * NTOK + q0 + qw)
                        nc.tensor.matmul(
                            ps_s[:kw, hh, q0:q0 + qw],
                            qkT[rows, KC + pair, ks],
                            qkT[rows, pair, qs],
                            start=True, stop=True,
                        )
                nc.scalar.activation(
                    exps[:kw, :, c_i, :], ps_s[:kw, :, :NTOK],
                    AF.Exp, scale=0.125,
                )
                feed()
            return exps


        def emit_attnv(it, pair, exps, coll):
            yraws = []
            for q0, qw in QT:
                for hh in range(2):
                    h = 2 * pair + hh
                    base = 32 * (2 * it + hh)
                    ps_y = at_ps.tile(
                        [P, 512], F32, tag="y", name="y", bufs=2
                    )[:65, :qw]
                    for c_i, (k0, kw) in enumerate(KCH):
                        nc.tensor.matmul(
                            ps_y,
                            v_aug[:kw, it * 5 + c_i, h, :],
                            exps[:kw, hh, c_i, q0:q0 + qw],
                            start=(c_i == 0), stop=(c_i == 4),
                        )
                    nc.vector.tensor_copy(
                        out=coll[base:base + 1, q0:q0 + qw], in_=ps_y[64:65, :]
                    )
                    if q0 == 0:
                        yr = at_sb.tile(
                            [D, NTOK], BF16, tag="yraw", name="yraw", bufs=4
                        )
                        yraws.append(yr)
                    else:
                        yr = yraws[hh]
                    nc.vector.tensor_copy(out=yr[:, q0:q0 + qw], in_=ps_y[0:D, :])
                    feed()
            return yraws

        def finish_group(pair, coll, yraws2):
            recf = at_sb.tile([97, NTOK], F32, tag="recf", name="recf", bufs=1)
            nc.vector.reciprocal_approx_fast(recf, coll)
            recbf = at_sb.tile([97, NTOK], BF16, tag="recbf", name="recbf", bufs=1)
            # DVE, not ACT: keeps the exp stream free of this dependency
            nc.vector.tensor_copy(out=recbf, in_=recf)
            for it in range(2):
                for hh in range(2):
                    base = 32 * (2 * it + hh)
                    for q0, qw in QT:
                        qs = slice(it * NTOK + q0, it * NTOK + q0 + qw)
                        ps_rb = at_ps.tile(
                            [P, 512], F32, tag="y", name="rb", bufs=2
                        )[:D, :qw]
                        nc.tensor.matmul(
                            ps_rb,
                            ones_pp[base:base + 1, 0:D],
                            recbf[base:base + 1, q0:q0 + qw],
                            start=True, stop=True,
                            tile_position=(base, 0),
                        )
                        nc.vector.tensor_tensor(
                            out=yT[hh * D:(hh + 1) * D, pair, qs],
                            in0=yraws2[it][hh][:, q0:q0 + qw],
                            in1=ps_rb, op=ALU.mult,
                        )

        # front: LN1 tiles interleaved with pair-0 q/k, v(half0) and even
        # pair-0 scores, so both PE and ACT ramp as early as possible
        wqk0 = {}
        for m in (0, KC):
            wqk0[m] = wqk_pool.tile([P, KC, P], BF16, tag="wqkb", name="wqkb")
            nc.gpsimd.dma_start(out=wqk0[m], in_=wqk_d[m])
        FTT = [(0, 192), (192, 193), (385, 385), (770, 384)]
        front_v = {0: [(0, 0, 0)],
                   1: [(0, 0, 1), (0, 0, 2)],
                   2: [(0, 0, 3), (0, 0, 4), (1, 0, 0)],
                   3: [(1, 0, 1), (1, 0, 2), (1, 0, 3), (1, 0, 4)]}
        exps_cur = [None, None]
        with ExitStack() as lnx:
            ln_sb = lnx.enter_context(tc.tile_pool(name="ln1sb", bufs=2))
            ln_ps = lnx.enter_context(
                tc.tile_pool(name="ln1ps", bufs=1, space="PSUM")
            )
            for t_idx, (t0, tw) in enumerate(FTT):
                ln_tile(xT, rows2a, t0, tw, ln_sb, ln_ps,
                        rb_ps=mm_ps, rb_tag="mm")
                for m in (0, KC):
                    emit_qk_tile(m, wqk0[m], t0, tw)
                for it, half, c_i in front_v[t_idx]:
                    v_quantum(it, half, c_i)()
                if t_idx == 2:
                    # item0 spans tokens 0..577 < 770, so its pair-0
                    # scores only need z/qkT through token 770
                    exps_cur[0] = emit_scores(0, 0)
            exps_cur[1] = emit_scores(1, 0)

        at_ps = s_attn.enter_context(
            tc.tile_pool(name="atps", bufs=1, space="PSUM")
        )
        wp_early = inner.enter_context(tc.tile_pool(name="wpe", bufs=1))
        wpb_early = []
        for m2 in range(4):
            wt = wp_early.tile([P, KC, P], BF16, tag=f"wpe{m2}", name="wpe")
            nc.sync.dma_start(out=wt, in_=wp_d[m2])
            wpb_early.append(wt)
        early_pj = []

        # one-pair lookahead: scores+exps for pair p+1 are emitted before
        # finish_group(p), so the scalar engine's exp stream never drains
        budget[0] = max(1, math.ceil(len(quanta) / 8))
        # one persistent coll: every writer/reader is on the in-order DVE
        # queue, so no per-pair buffer rotation or re-memset is needed
        coll = at_sb.tile([97, NTOK], F32, tag="coll", name="coll", bufs=1)
        nc.vector.memset(coll, 1.0)
        for pair in range(H // 2):
            drain(2 * pair + 1)
            yr0 = emit_attnv(0, pair, exps_cur[0], coll)
            yr1 = emit_attnv(1, pair, exps_cur[1], coll)
            if pair < H // 2 - 1:
                drain(2 * pair + 2)
                cur[0] = pair + 1
                budget[0] = max(1, math.ceil(len(quanta) / (7 - pair)))
                exps_cur = [emit_scores(0, pair + 1), emit_scores(1, pair + 1)]
            else:
                while quanta:
                    quanta.popleft()[1]()
                # partial proj groups for (tile0, m=0,1) fill the last
                # pair's exp-wait stalls; ci=7 lands after finish_group(7)
                t0e, twe = TT[0]
                tse = slice(t0e, t0e + twe)
                # m0/m1 use the idle quanta banks, m2/m3 the idle score banks
                for m2 in (0, 1):
                    early_pj.append(mm_ps.tile(
                        [P, 512], F32, tag="mm", name="pj0", bufs=2
                    )[:, :twe])
                ps_sc = mm_ps.tile(
                    [P, 2, 1024], F32, tag="score", name="pj0b", bufs=1
                )
                early_pj.append(ps_sc[:, 0, :twe])
                early_pj.append(ps_sc[:, 1, :twe])
                for m2 in range(4):
                    for ci in range(KC - 1):
                        nc.tensor.matmul(
                            early_pj[m2], wpb_early[m2][:, ci, :],
                            yT[:, ci, tse],
                            start=(ci == 0), stop=False,
                        )
            finish_group(pair, coll, [yr0, yr1])
        t0e, twe = TT[0]
        tse = slice(t0e, t0e + twe)
        for m2 in range(4):
            nc.tensor.matmul(
                early_pj[m2], wpb_early[m2][:, KC - 1, :], yT[:, KC - 1, tse],
                start=False, stop=True,
            )
            nc.vector.scalar_tensor_tensor(
                xT[:, m2, tse], early_pj[m2], pb[:, m2:m2 + 1], xT[:, m2, tse],
                ALU.add, ALU.add,
            )

        s_attn.close()
        inner.close()  # free qkT / v_aug / exps / wv / wqk before proj

        # ---------------- proj + residual + LN2 (per tile) ----------------
        w2_pool = top.enter_context(tc.tile_pool(name="w2pool", bufs=3))
        w2bs = {}
        for m in (0, 1):
            w2bs[m] = w2_pool.tile([P, MH, P], BF16, tag="w2b", name="w2b")
            nc.sync.dma_start(out=w2bs[m], in_=w2_d[m])
        ln2_sb = top.enter_context(tc.tile_pool(name="ln2sb", bufs=2))
        ln2_ps = top.enter_context(
            tc.tile_pool(name="ln2ps", bufs=1, space="PSUM")
        )
        with ExitStack() as pjx:
            pj_ps = pjx.enter_context(
                tc.tile_pool(name="pjps", bufs=2, space="PSUM")
            )
            wp_pool = pjx.enter_context(tc.tile_pool(name="wpp", bufs=3))
            for t0, tw in TT:
                ts = slice(t0, t0 + tw)
                for m in range(KC):
                    if t0 == 0 and m < 4:
                        continue  # done early, at the attention tail
                    wpb = wp_pool.tile([P, KC, P], BF16, tag="wpb", name="wpb")
                    nc.gpsimd.dma_start(out=wpb, in_=wp_d[m])
                    ps = pj_ps.tile([P, 512], F32, tag="z", name="z", bufs=2)[:, :tw]
                    for ci in range(KC):
                        nc.tensor.matmul(
                            ps, wpb[:, ci, :], yT[:, ci, ts],
                            start=(ci == 0), stop=(ci == KC - 1),
                        )
                    # x2 = (z + bp) + x   (in place into xT)
                    nc.vector.scalar_tensor_tensor(
                        xT[:, m, ts], ps, pb[:, m:m + 1], xT[:, m, ts],
                        ALU.add, ALU.add,
                    )
                if t0 == 0:
                    # only tile0's LN2 gates fc1-A; the rest run under it
                    ln_tile(xT, rows2b, t0, tw, ln2_sb, ln2_ps)


        # ------------------------- MLP -------------------------
        with ExitStack() as mx:
            mlpc = mx.enter_context(tc.tile_pool(name="mlpc", bufs=1))
            h_p = mx.enter_context(tc.tile_pool(name="hpool", bufs=1))
            ub1 = mlpc.tile([P, MH, P], BF16)
            nc.gpsimd.dma_start(out=ub1, in_=ub1_d[:])
            hT = h_p.tile([P, MH, T], BF16)

            # fc1 in two tile groups so it can start right after LN2(t0);
            # one shared weight pool with prefetch across the A/B boundary
            with ExitStack() as f1x:
                w1_pool = f1x.enter_context(tc.tile_pool(name="w1pool", bufs=4))
                f1_ps = f1x.enter_context(
                    tc.tile_pool(name="f1ps", bufs=4, space="PSUM")
                )
                steps = [(TT[:1], mh) for mh in range(MH)]
                steps += [(TT[1:], mh) for mh in range(MH)]
                w1bs = {}

                def fetch_w1(i):
                    if i < len(steps) and i not in w1bs:
                        w1bs[i] = w1_pool.tile(
                            [P, KC, P], BF16, tag="w1b", name="w1b"
                        )
                        nc.gpsimd.dma_start(out=w1bs[i], in_=w1_d[steps[i][1]])

                for i, (tgroup, mh) in enumerate(steps):
                    if i == 4:
                        # deferred LN2 tiles run underneath fc1-A
                        ln_tile(xT, rows2b, *TT[1], ln2_sb, ln2_ps)
                    elif i == 8:
                        ln_tile(xT, rows2b, *TT[2], ln2_sb, ln2_ps)
                    fetch_w1(i)
                    fetch_w1(i + 1)
                    fetch_w1(i + 2)
                    w1b = w1bs.pop(i)
                    for t0, tw in tgroup:
                        ts = slice(t0, t0 + tw)
                        ps = f1_ps.tile([P, 512], F32, tag="h", name="h")[:, :tw]
                        nc.tensor.matmul(
                            ps, ub1[:, mh, :], rows2b[:, ts],
                            start=True, stop=False,
                        )
                        for ci in range(KC):
                            nc.tensor.matmul(
                                ps, w1b[:, ci, :], z[:, ci, ts],
                                start=False, stop=(ci == KC - 1),
                            )
                        nc.scalar.activation(hT[:, mh, ts], ps, AF.Gelu)

            with ExitStack() as f2x:
                f2_ps = f2x.enter_context(
                    tc.tile_pool(name="f2ps", bufs=2, space="PSUM")
                )
                o_pool = f2x.enter_context(tc.tile_pool(name="opool", bufs=3))
                for m in range(KC):
                    if m not in w2bs:
                        w2bs[m] = w2_pool.tile([P, MH, P], BF16, tag="w2b", name="w2b")
                        nc.sync.dma_start(out=w2bs[m], in_=w2_d[m])
                    if m + 1 < KC:
                        # prefetch next block so its 1MB DMA hides under MMs
                        w2bs[m + 1] = w2_pool.tile(
                            [P, MH, P], BF16, tag="w2b", name="w2b"
                        )
                        nc.sync.dma_start(out=w2bs[m + 1], in_=w2_d[m + 1])
                    w2b = w2bs[m]
                    for t0, tw in TT:
                        ts = slice(t0, t0 + tw)
                        ps = f2_ps.tile([P, 512], F32, tag="o", name="o")[:, :tw]
                        for kh in range(MH):
                            nc.tensor.matmul(
                                ps, w2b[:, kh, :], hT[:, kh, ts],
                                start=(kh == 0), stop=(kh == MH - 1),
                            )
                        osb = o_pool.tile([P, 512], F32, tag="osb", name="osb")[:, :tw]
                        nc.vector.scalar_tensor_tensor(
                            osb, ps, b2[:, m:m + 1], xT[:, m, ts],
                            ALU.add, ALU.add,
                        )
                        nc.sync.dma_start(out=out_d[:, m, ts], in_=osb)

    nc.compile()
    return nc


def _program():
    global _NC
    if _NC is None:
        _NC = _build()
    return _NC


def host_inputs(x, w_qkv, b_qkv, w_proj, b_proj, ln1_g, ln1_b, ln2_g, ln2_b,
                w_fc1, b_fc1, w_fc2, b_fc2):
    """Fold LN gains into weights and build the per-core input maps."""
    bf = ml_dtypes.bfloat16
    x = np.asarray(x, dtype=np.float32)
    B = x.shape[0]

    # feature-major x, chunked: [B, P, KC, NTOK]
    xTt = np.ascontiguousarray(
        x.transpose(0, 2, 1).reshape(B, KC, P, NTOK).transpose(0, 2, 1, 3)
    )

    wqk_f = w_qkv[:2 * C] * ln1_g[None, :]
    wv_f = w_qkv[2 * C:] * ln1_g[None, :]
    w1_f = w_fc1 * ln2_g[None, :]

    wqkT = wqk_f.T.reshape(KC, P, 2 * KC, P).transpose(2, 1, 0, 3)
    wqkT = np.ascontiguousarray(wqkT).astype(bf)
    wvT = np.ascontiguousarray(
        wv_f.T.reshape(KC, P, C).transpose(1, 0, 2)).astype(bf)
    wpT = w_proj.T.reshape(KC, P, KC, P).transpose(2, 1, 0, 3)
    wpT = np.ascontiguousarray(wpT).astype(bf)
    w1T = w1_f.T.reshape(KC, P, MH, P).transpose(2, 1, 0, 3)
    w1T = np.ascontiguousarray(w1T).astype(bf)
    w2T = w_fc2.T.reshape(MH, P, KC, P).transpose(2, 1, 0, 3)
    w2T = np.ascontiguousarray(w2T).astype(bf)

    def pad_rows(u, b, shape):
        # row 0 = u, row 32 = b, zeros elsewhere (full-128-row correction
        # operands keep the tensor engine's weight-load pull-ahead alive)
        out = np.zeros((P,) + shape[1:], dtype=np.float32)
        out[0] = u.reshape(shape[1:])
        out[32] = b.reshape(shape[1:])
        return np.ascontiguousarray(out).astype(bf)

    uqk = -wqk_f.sum(axis=1)
    bqk = b_qkv[:2 * C] + w_qkv[:2 * C] @ ln1_b
    ubqk = pad_rows(uqk, bqk, (P, 2 * KC, P))
    uv = -wv_f.sum(axis=1)
    bv = b_qkv[2 * C:] + w_qkv[2 * C:] @ ln1_b
    uvbv = pad_rows(uv, bv, (P, C))
    u1 = -w1_f.sum(axis=1)
    b1 = b_fc1 + w_fc1 @ ln2_b
    ub1 = pad_rows(u1, b1, (P, MH, P))

    pb = np.ascontiguousarray(b_proj.reshape(KC, P).T).astype(np.float32)
    b2a = np.ascontiguousarray(b_fc2.reshape(KC, P).T).astype(np.float32)

    shared = dict(
        wqk=wqkT, wv=wvT, wp=wpT, w1=w1T, w2=w2T,
        ubqk=ubqk, uvbv=uvbv, ub1=ub1, pb=pb, b2=b2a,
    )
    in_maps = []
    for core in range(N_CORES):
        xc = np.concatenate([xTt[2 * core], xTt[2 * core + 1]], axis=2)
        in_maps.append(dict(xT=np.ascontiguousarray(xc), **shared))
    return in_maps


def unshard(results, B):
    out = np.empty((B, NTOK, C), dtype=np.float32)
    for core in range(N_CORES):
        o = results[core]["outT"]  # [P, KC, T]
        full = o.transpose(1, 0, 2).reshape(C, T)
        out[2 * core] = full[:, :NTOK].T
        out[2 * core + 1] = full[:, NTOK:].T
    return out


def kernel(x, w_qkv, b_qkv, w_proj, b_proj, ln1_g, ln1_b, ln2_g, ln2_b,
           w_fc1, b_fc1, w_fc2, b_fc2, _trace=False, _tmpdir=None):
    global LAST_EXEC_NS
    in_maps = host_inputs(
        x, w_qkv, b_qkv, w_proj, b_proj, ln1_g, ln1_b, ln2_g, ln2_b,
        w_fc1, b_fc1, w_fc2, b_fc2,
    )
    nc = _program()
    res = run_bass_kernel_spmd(
        nc, in_maps, list(range(N_CORES)), trace=_trace, tmpdir=_tmpdir
    )
    LAST_EXEC_NS = res.exec_time_ns
    return unshard(res.results, np.asarray(x).shape[0])
